# revision 12
# baseline (speedup 1.0000x reference)
"""Chamfer loss kernel for Trainium2 (8 NeuronCores, data-parallel over batch).

Math: for each sample b,
    loss[b] = sum_n o_w[b,n] * min_m(masked dist)^2 + sum_m t_w[b,m] * min_n(...)^2
with dist the Euclidean distance matrix between outputs[b] ([N,D]) and
targets[b] ([M,D]), masked entries (o_w==0 or t_w==0) excluded from the mins.

Key transformations (host-side marshalling, device does the O(N*M*D) work):
  * Weights are {0,1}.  Rows with o_w==0 and cols with t_w==0 provably do not
    affect the loss (their min is multiplied by 0; for the other direction the
    mask excludes them).  So each sample is compacted to its live rows/cols
    (~half), shrinking the N x M work 4x.
  * The squared-distance matrix is computed NEGATED via one augmented matmul:
        e[n,m] = 2*a.b - |a|^2 - |b|^2  = -d2[n,m]
    folding the norm and mask terms into the contraction dimension.  fp16
    inputs are split hi/lo so products are exact (fp32-quality accumulation
    in PSUM):  K = 16*3 (a_hi*b_hi, a_lo*b_hi, a_hi*b_lo) + 4 norm channels.
  * min_m d2 = -max_m e, so row/col minima become maxima.  ScalarE casts each
    PSUM tile to fp16 into a [128, nt, m_pad] SBUF buffer; VectorE then runs
    multi-tile 2x-mode tensor_tensor max fold-trees: along the free (m) axis
    for row maxima (finished by one 1x reduce of the folded remnant) and
    along the tile (i) axis for column maxima (finished by PE transposes and
    a cross-partition reduce).
  * min^2 with the reference's max(d2,0) clamp is relu(-rowmax); weighting and
    sums are DVE elementwise ops plus a PE ones-matmul partition sum.
"""

import math
import os

import numpy as np

NCORES = 8
# "masked" magnitude; far beyond any real |e| (<~500) yet small enough that
# even a doubly-masked entry (-2*HUGE) stays finite in fp16 (max 65504).
HUGE = 30000.0

_PROGRAM_CACHE = {}


def _chunk_widths(m_pad):
    """Split m_pad (multiple of 64) into matmul free-dim chunks of <=512."""
    widths = []
    rem = m_pad
    while rem > 0:
        w = min(512, rem)
        widths.append(w)
        rem -= w
    return widths


def _fold_pairs(n):
    """Yield (half, rest) steps halving n down to 1 (handles odd n)."""
    steps = []
    while n > 1:
        half = n // 2
        steps.append((half, n))
        n -= half
    return steps


def _build_program(nt, m_pad, n_samples):
    import concourse.bacc as bacc
    import concourse.mybir as mybir
    from concourse import tile

    f16 = mybir.dt.float16
    f32 = mybir.dt.float32
    Alu = mybir.AluOpType
    Act = mybir.ActivationFunctionType
    Axis = mybir.AxisListType

    K = 52
    n_pad = nt * 128
    widths = _chunk_widths(m_pad)
    mblocks = [128] * (m_pad // 128)
    if m_pad % 128:
        mblocks.append(m_pad % 128)

    nc = bacc.Bacc("TRN2", target_bir_lowering=False, debug=False,
                   num_devices=NCORES)

    a_in = nc.dram_tensor("a_aug", [n_samples, K, n_pad], f16,
                          kind="ExternalInput")
    b_in = nc.dram_tensor("b_aug", [n_samples, K, m_pad], f16,
                          kind="ExternalInput")
    w_in = nc.dram_tensor("w", [n_samples, 128, 32], f32,
                          kind="ExternalInput")
    i_in = nc.dram_tensor("ident", [128, 128], f16, kind="ExternalInput")
    o_in = nc.dram_tensor("ones", [128, 1], f32, kind="ExternalInput")
    y_out = nc.dram_tensor("y", [1, n_samples], f32, kind="ExternalOutput")

    with tile.TileContext(nc) as tc:
        with (
            tc.tile_pool(name="const", bufs=1) as constp,
            tc.tile_pool(name="ab", bufs=2) as abp,
            tc.tile_pool(name="buf", bufs=2) as bufp,
            tc.tile_pool(name="small", bufs=3) as smallp,
            tc.tile_pool(name="ps", bufs=2, space="PSUM") as psp,
            tc.tile_pool(name="pst", bufs=1, space="PSUM") as pstp,
            tc.tile_pool(name="pss", bufs=1, space="PSUM") as pssp,
        ):
            ident = constp.tile([128, 128], f16)
            nc.sync.dma_start(ident[:], i_in[:, :])
            ones = constp.tile([128, 1], f32)
            nc.sync.dma_start(ones[:], o_in[:, :])
            out_sb = constp.tile([1, n_samples], f32)

            # PE HAM warm-up: ~4us of dense matmuls on zeroed SBUF so the
            # clock gate opens before the real stream begins.
            warm = constp.tile([64, 512], f16)
            nc.vector.memset(warm[:], 0.0)
            pwarm = psp.tile([128, 512], f32, tag="ps")
            for _ in range(14):
                nc.tensor.matmul(pwarm[:], warm[:, 0:128], warm[:],
                                 start=True, stop=True)


            for s in range(n_samples):
                # Operands duplicated at partition offsets 0 and 64 so
                # consecutive n-tiles use disjoint PE row-groups (hides
                # LDWEIGHTS behind in-flight matmuls).
                ah = abp.tile([64 + K, n_pad], f16, tag="ah")
                nc.sync.dma_start(ah[0:K, :], a_in[s, :, :])
                nc.sync.dma_start(ah[64:64 + K, :], a_in[s, :, :])
                bh = abp.tile([64 + K, m_pad], f16, tag="bh")
                nc.sync.dma_start(bh[0:K, :], b_in[s, :, :])
                nc.sync.dma_start(bh[64:64 + K, :], b_in[s, :, :])
                wt = abp.tile([128, 32], f32, tag="wt")
                nc.sync.dma_start(wt[:], w_in[s, :, :])

                buf = bufp.tile([128, nt, m_pad], f16, tag="buf")
                x = smallp.tile([128, 32], f32, tag="x")
                nc.vector.memset(x[:], 0.0)

                for i in range(nt):
                    po = 64 * (i % 2)  # ping-pong PE row-group offset
                    ps = psp.tile([128, m_pad], f32, tag="ps")
                    off = 0
                    for wc in widths:
                        nc.tensor.matmul(
                            ps[:, off:off + wc],
                            ah[po:po + K, i * 128:(i + 1) * 128],
                            bh[po:po + K, off:off + wc],
                            start=True, stop=True,
                        )
                        off += wc
                    nc.scalar.activation(buf[:, i, :], ps[:], Act.Copy)

                # Row maxima: fold-tree along m (multi-tile 2x tensor_tensor),
                # then one 1x reduce of the remnant -> X[:, 0:nt].
                # rf holds the fold chain; first fold reads buf (must precede
                # the in-place column folds below).
                w0 = m_pad // 2
                rf = bufp.tile([128, nt, w0], f16, tag="rf")
                nc.vector.tensor_tensor(
                    rf[:, :, :], buf[:, :, 0:w0], buf[:, :, w0:2 * w0],
                    op=Alu.max)
                # (m_pad even; if odd-width remnant existed it would need an
                # extra lone fold -- m_pad is a multiple of 64 so w0*2==m_pad)
                w = w0
                while w > 160 and w % 2 == 0:
                    h = w // 2
                    nc.vector.tensor_tensor(
                        rf[:, :, 0:h], rf[:, :, 0:h], rf[:, :, h:w],
                        op=Alu.max)
                    w = h
                nc.vector.tensor_reduce(
                    x[:, 0:nt], rf[:, :, 0:w], axis=Axis.X, op=Alu.max)

                # Column maxima: fold tiles together in place along i
                # (preserves m identity), then PE-transpose 128-blocks and
                # reduce the partition axis (now free).
                for half, n in _fold_pairs(nt):
                    nc.vector.tensor_tensor(
                        buf[:, 0:half, :], buf[:, 0:half, :],
                        buf[:, n - half:n, :], op=Alu.max)
                imax = buf[:, 0, :]

                # Group full 128-wide blocks by 4; a partial tail block gets
                # its own group so every reduce only reads PSUM partitions
                # its transposes actually wrote.
                nfull = len([b for b in mblocks if b == 128])
                groups = [mblocks[:nfull][i:i + 4]
                          for i in range(0, nfull, 4)]
                if nfull < len(mblocks):
                    groups.append([mblocks[-1]])
                bi = 0
                off = 0
                for grp in groups:
                    ngrp = len(grp)
                    bw0 = grp[-1]  # only the last block can be partial
                    pst = pstp.tile([128, 512], f16, tag="pst")
                    goff = 0
                    for g, bw in enumerate(grp):
                        nc.tensor.transpose(
                            pst[0:bw, g * 128:(g + 1) * 128],
                            imax[:, off + goff:off + goff + bw],
                            ident[:])
                        goff += bw
                    pst3 = pst[0:bw0, 0:ngrp * 128].rearrange(
                        "p (g q) -> p g q", q=128)
                    nc.vector.tensor_reduce(
                        x[0:bw0, 16 + bi:16 + bi + ngrp], pst3,
                        axis=Axis.X, op=Alu.max)
                    bi += ngrp
                    off += goff

                # loss terms: v = relu(-x) (finite by construction); then
                # s = sum_f v*w and a PE ones-matmul partition sum.
                v = smallp.tile([128, 32], f32, tag="v")
                nc.vector.tensor_scalar(
                    v[:], x[:], -1.0, 0.0, op0=Alu.mult, op1=Alu.max)
                vw = smallp.tile([128, 32], f32, tag="vw")
                nc.vector.tensor_tensor(vw[:], v[:], wt[:], op=Alu.mult)
                ssum = smallp.tile([128, 1], f32, tag="ssum")
                nc.vector.tensor_reduce(
                    ssum[:], vw[:], axis=Axis.X, op=Alu.add)
                pss = pssp.tile([1, 1], f32, tag="pssum")
                nc.tensor.matmul(pss[:], ssum[:], ones[:], start=True,
                                 stop=True)
                nc.scalar.activation(out_sb[:, s:s + 1], pss[:], Act.Copy)

            nc.sync.dma_start(y_out[:, :], out_sb[:])

    nc.compile()
    return nc


def _split16(x):
    """fp32(-ish) array -> (hi, lo) float16 pair with x ~= hi + lo."""
    hi = x.astype(np.float16)
    lo = (x.astype(np.float64) - hi.astype(np.float64)).astype(np.float16)
    return hi, lo


def _prep_sample(a_live, b_live, n_pad, m_pad):
    """Build augmented operand matrices for one sample.

    Returns (A [52, n_pad] f16, B [52, m_pad] f16) so that
    (A.T @ B)[n, m] = 2*a.b - |a|^2 - |b|^2   (= -d2, ~fp32 precision),
    with padded rows/cols pushed to ~-HUGE.
    """
    n_live, d = a_live.shape
    m_live = b_live.shape[0]
    assert d == 16

    a_hi, a_lo = _split16(a_live)
    b_hi, b_lo = _split16(b_live)
    a2 = np.sum(a_live.astype(np.float64) ** 2, axis=1)
    b2 = np.sum(b_live.astype(np.float64) ** 2, axis=1)
    a2n_hi, a2n_lo = _split16(-a2)
    b2_hi, b2_lo = _split16(b2)

    A = np.zeros((52, n_pad), np.float16)
    A[0:16, :n_live] = (np.float16(2) * a_hi).T
    A[16:32, :n_live] = (np.float16(2) * a_lo).T
    A[32:48, :n_live] = (np.float16(2) * a_hi).T
    A[48, :] = np.float16(-1)
    A[49, :] = np.float16(-1)
    A[50, :n_live] = a2n_hi
    A[50, n_live:] = np.float16(-HUGE)
    A[51, :n_live] = a2n_lo

    B = np.zeros((52, m_pad), np.float16)
    B[0:16, :m_live] = b_hi.T
    B[16:32, :m_live] = b_hi.T
    B[32:48, :m_live] = b_lo.T
    B[48, :m_live] = b2_hi
    B[48, m_live:] = np.float16(HUGE)
    B[49, :m_live] = b2_lo
    B[50, :] = np.float16(1)
    B[51, :] = np.float16(1)
    return A, B


def kernel(o_weights, outputs, t_weights, targets):
    from concourse.bass_utils import run_bass_kernel_spmd

    o_weights = np.asarray(o_weights, np.float32)
    t_weights = np.asarray(t_weights, np.float32)
    outputs = np.asarray(outputs, np.float32)
    targets = np.asarray(targets, np.float32)

    B, N, D = outputs.shape
    M = targets.shape[1]
    assert B % NCORES == 0, f"batch {B} not divisible by {NCORES}"
    n_samples = B // NCORES

    o_idx = [np.nonzero(o_weights[b])[0] for b in range(B)]
    t_idx = [np.nonzero(t_weights[b])[0] for b in range(B)]
    max_n = max(1, max(len(ix) for ix in o_idx))
    max_m = max(1, max(len(ix) for ix in t_idx))
    nt = math.ceil(max_n / 128)
    n_pad = nt * 128
    m_pad = 64 * math.ceil(max_m / 64)

    key = (nt, m_pad, n_samples)
    if key not in _PROGRAM_CACHE:
        _PROGRAM_CACHE[key] = _build_program(nt, m_pad, n_samples)
    nc = _PROGRAM_CACHE[key]

    a_aug = np.zeros((B, 52, n_pad), np.float16)
    b_aug = np.zeros((B, 52, m_pad), np.float16)
    w_arr = np.zeros((B, 128, 32), np.float32)
    nblk = math.ceil(m_pad / 128)
    for b in range(B):
        n_live, m_live = len(o_idx[b]), len(t_idx[b])
        a_aug[b], b_aug[b] = _prep_sample(
            outputs[b][o_idx[b]], targets[b][t_idx[b]], n_pad, m_pad)
        nn = np.arange(n_pad) < n_live
        w_arr[b, :, 0:nt] = nn.reshape(nt, 128).T
        mm = np.zeros(nblk * 128, bool)
        mm[:m_pad] = np.arange(m_pad) < m_live
        w_arr[b, :, 16:16 + nblk] = mm.reshape(nblk, 128).T

    ident = np.eye(128, dtype=np.float16)
    ones = np.ones((128, 1), np.float32)
    in_maps = []
    for k in range(NCORES):
        sl = slice(k * n_samples, (k + 1) * n_samples)
        in_maps.append({
            "a_aug": a_aug[sl], "b_aug": b_aug[sl], "w": w_arr[sl],
            "ident": ident, "ones": ones,
        })

    trace = bool(os.environ.get("CHAMFER_TRACE"))
    kw = {}
    if trace:
        kw = {"trace": True,
              "tmpdir": os.environ.get("CHAMFER_TRACE_DIR") or None}
    res = run_bass_kernel_spmd(nc, in_maps, list(range(NCORES)), **kw)
    if trace and res.exec_time_ns is not None:
        print(f"HW exec time: {res.exec_time_ns} ns")

    out = np.empty((B,), np.float32)
    for k in range(NCORES):
        out[k * n_samples:(k + 1) * n_samples] = res.results[k]["y"][0]
    return out


# revision 16
# speedup vs baseline: 1.0847x; 1.0847x over previous
"""Chamfer loss kernel for Trainium2 (8 NeuronCores, data-parallel over batch).

Math: for each sample b,
    loss[b] = sum_n o_w[b,n] * min_m(masked dist)^2 + sum_m t_w[b,m] * min_n(...)^2
with dist the Euclidean distance matrix between outputs[b] ([N,D]) and
targets[b] ([M,D]), masked entries (o_w==0 or t_w==0) excluded from the mins.

Key transformations (host-side marshalling, device does the O(N*M*D) work):
  * Weights are {0,1}.  Rows with o_w==0 and cols with t_w==0 provably do not
    affect the loss (their min is multiplied by 0; for the other direction the
    mask excludes them).  So each sample is compacted to its live rows/cols
    (~half), shrinking the N x M work 4x.
  * The squared-distance matrix is computed NEGATED via one augmented matmul:
        e[n,m] = 2*a.b - |a|^2 - |b|^2  = -d2[n,m]
    folding the norm and mask terms into the contraction dimension.  fp16
    inputs are split hi/lo so products are exact (fp32-quality accumulation
    in PSUM):  K = 16*3 (a_hi*b_hi, a_lo*b_hi, a_hi*b_lo) + 4 norm channels.
  * min_m d2 = -max_m e, so row/col minima become maxima.  ScalarE casts each
    PSUM tile to fp16 into a [128, nt, m_pad] SBUF buffer; VectorE then runs
    multi-tile 2x-mode tensor_tensor max fold-trees: along the free (m) axis
    for row maxima (finished by one 1x reduce of the folded remnant) and
    along the tile (i) axis for column maxima (finished by PE transposes and
    a cross-partition reduce).
  * min^2 with the reference's max(d2,0) clamp is relu(-rowmax); weighting and
    sums are DVE elementwise ops plus a PE ones-matmul partition sum.
"""

import math
import os

import numpy as np

NCORES = 8
# "masked" magnitude; far beyond any real |e| (<~500) yet small enough that
# even a doubly-masked entry (-2*HUGE) stays finite in fp16 (max 65504).
HUGE = 30000.0

_PROGRAM_CACHE = {}


def _chunk_widths(m_pad):
    """Split m_pad (multiple of 64) into matmul free-dim chunks of <=512."""
    widths = []
    rem = m_pad
    while rem > 0:
        w = min(512, rem)
        widths.append(w)
        rem -= w
    return widths


def _fold_pairs(n):
    """Yield (half, rest) steps halving n down to 1 (handles odd n)."""
    steps = []
    while n > 1:
        half = n // 2
        steps.append((half, n))
        n -= half
    return steps


def _build_program(nt, m_pad, n_samples):
    import concourse.bacc as bacc
    import concourse.mybir as mybir
    from concourse import tile

    f16 = mybir.dt.float16
    f32 = mybir.dt.float32
    Alu = mybir.AluOpType
    Act = mybir.ActivationFunctionType
    Axis = mybir.AxisListType

    K = 52
    n_pad = nt * 128
    widths = _chunk_widths(m_pad)
    mblocks = [128] * (m_pad // 128)
    if m_pad % 128:
        mblocks.append(m_pad % 128)

    nc = bacc.Bacc("TRN2", target_bir_lowering=False, debug=False,
                   num_devices=NCORES)

    a_in = nc.dram_tensor("a_aug", [n_samples, K, n_pad], f16,
                          kind="ExternalInput")
    b_in = nc.dram_tensor("b_aug", [n_samples, K, m_pad], f16,
                          kind="ExternalInput")
    w_in = nc.dram_tensor("w", [n_samples, 128, 32], f32,
                          kind="ExternalInput")
    i_in = nc.dram_tensor("ident", [128, 128], f16, kind="ExternalInput")
    o_in = nc.dram_tensor("ones", [128, 1], f32, kind="ExternalInput")
    y_out = nc.dram_tensor("y", [1, n_samples], f32, kind="ExternalOutput")

    with tile.TileContext(nc) as tc:
        with (
            tc.tile_pool(name="const", bufs=1) as constp,
            tc.tile_pool(name="ab", bufs=2) as abp,
            tc.tile_pool(name="buf", bufs=2) as bufp,
            tc.tile_pool(name="small", bufs=3) as smallp,
            tc.tile_pool(name="ps", bufs=2, space="PSUM") as psp,
            tc.tile_pool(name="pst", bufs=2, space="PSUM") as pstp,
        ):
            ident = constp.tile([128, 128], f16)
            nc.sync.dma_start(ident[:], i_in[:, :])
            ones = constp.tile([128, 1], f32)
            nc.sync.dma_start(ones[:], o_in[:, :])
            out_sb = constp.tile([1, n_samples], f32)



            for s in range(n_samples):
                # Operands duplicated at partition offsets 0 and 64 so
                # consecutive n-tiles use disjoint PE row-groups (hides
                # LDWEIGHTS behind in-flight matmuls).
                ah = abp.tile([64 + K, n_pad], f16, tag="ah")
                nc.sync.dma_start(ah[0:K, :], a_in[s, :, :])
                nc.sync.dma_start(ah[64:64 + K, :], a_in[s, :, :])
                bh = abp.tile([64 + K, m_pad], f16, tag="bh")
                nc.sync.dma_start(bh[0:K, :], b_in[s, :, :])
                nc.sync.dma_start(bh[64:64 + K, :], b_in[s, :, :])
                wt = abp.tile([128, 32], f32, tag="wt")
                nc.sync.dma_start(wt[:], w_in[s, :, :])

                buf = bufp.tile([128, nt, m_pad], f16, tag="buf")
                x = smallp.tile([128, 32], f32, tag="x")
                nc.vector.memset(x[:], 0.0)

                w0 = m_pad // 2
                rf = bufp.tile([128, nt, w0], f16, tag="rf")
                incremental = (s == 0)
                if incremental:
                    colacc = bufp.tile([128, m_pad], f16, tag="colacc")

                for i in range(nt):
                    po = 64 * (i % 2)  # ping-pong PE row-group offset
                    ps = psp.tile([128, m_pad], f32, tag="ps")
                    off = 0
                    for wc in widths:
                        nc.tensor.matmul(
                            ps[:, off:off + wc],
                            ah[po:po + K, i * 128:(i + 1) * 128],
                            bh[po:po + K, off:off + wc],
                            start=True, stop=True,
                        )
                        off += wc
                    nc.scalar.activation(buf[:, i, :], ps[:], Act.Copy)
                    if incremental:
                        # Per-tile folds right after each cast: fills the
                        # pipeline-ramp bubble on the first sample.
                        nc.vector.tensor_tensor(
                            rf[:, i, :], buf[:, i, 0:w0],
                            buf[:, i, w0:m_pad], op=Alu.max)
                        if i == 1:
                            nc.vector.tensor_tensor(
                                colacc[:], buf[:, 0, :], buf[:, 1, :],
                                op=Alu.max)
                        elif i > 1:
                            nc.vector.tensor_tensor(
                                colacc[:], colacc[:], buf[:, i, :],
                                op=Alu.max)

                # Row maxima: fold-tree along m (multi-tile 2x tensor_tensor),
                # then one 1x reduce of the remnant -> X[:, 0:nt].
                if not incremental:
                    # first fold reads all of buf (must precede the in-place
                    # column folds below).
                    nc.vector.tensor_tensor(
                        rf[:, :, :], buf[:, :, 0:w0], buf[:, :, w0:2 * w0],
                        op=Alu.max)
                w = w0
                while w > 100 and w % 2 == 0:
                    h = w // 2
                    nc.vector.tensor_tensor(
                        rf[:, :, 0:h], rf[:, :, 0:h], rf[:, :, h:w],
                        op=Alu.max)
                    w = h
                nc.vector.tensor_reduce(
                    x[:, 0:nt], rf[:, :, 0:w], axis=Axis.X, op=Alu.max)

                # Column maxima: fold tiles together in place along i
                # (preserves m identity), then PE-transpose 128-blocks and
                # reduce the partition axis (now free).
                if incremental:
                    imax = colacc[:]
                else:
                    for half, n in _fold_pairs(nt):
                        nc.vector.tensor_tensor(
                            buf[:, 0:half, :], buf[:, 0:half, :],
                            buf[:, n - half:n, :], op=Alu.max)
                    imax = buf[:, 0, :]

                # Group full 128-wide blocks by 4; a partial tail block gets
                # its own group so every reduce only reads PSUM partitions
                # its transposes actually wrote.
                nfull = len([b for b in mblocks if b == 128])
                groups = [mblocks[:nfull][i:i + 4]
                          for i in range(0, nfull, 4)]
                if nfull < len(mblocks):
                    groups.append([mblocks[-1]])
                bi = 0
                off = 0
                for grp in groups:
                    ngrp = len(grp)
                    bw0 = grp[-1]  # only the last block can be partial
                    pst = pstp.tile([128, 512], f16, tag="pst")
                    goff = 0
                    for g, bw in enumerate(grp):
                        nc.tensor.transpose(
                            pst[0:bw, g * 128:(g + 1) * 128],
                            imax[:, off + goff:off + goff + bw],
                            ident[:])
                        goff += bw
                    pst3 = pst[0:bw0, 0:ngrp * 128].rearrange(
                        "p (g q) -> p g q", q=128)
                    nc.vector.tensor_reduce(
                        x[0:bw0, 16 + bi:16 + bi + ngrp], pst3,
                        axis=Axis.X, op=Alu.max)
                    bi += ngrp
                    off += goff

                # loss terms: v = relu(-x) (finite by construction); then
                # s = sum_f v*w and a PE ones-matmul partition sum.
                v = smallp.tile([128, 32], f32, tag="v")
                nc.vector.tensor_scalar(
                    v[:], x[:], -1.0, 0.0, op0=Alu.mult, op1=Alu.max)
                vw = smallp.tile([128, 32], f32, tag="vw")
                nc.vector.tensor_tensor(vw[:], v[:], wt[:], op=Alu.mult)
                ssum = smallp.tile([128, 1], f32, tag="ssum")
                nc.vector.tensor_reduce(
                    ssum[:], vw[:], axis=Axis.X, op=Alu.add)
                pss = pstp.tile([1, 1], f32, tag="pst")
                nc.tensor.matmul(pss[:], ssum[:], ones[:], start=True,
                                 stop=True)
                nc.scalar.activation(out_sb[:, s:s + 1], pss[:], Act.Copy)

            nc.sync.dma_start(y_out[:, :], out_sb[:])

    nc.compile()
    return nc


def _split16(x):
    """fp32(-ish) array -> (hi, lo) float16 pair with x ~= hi + lo."""
    hi = x.astype(np.float16)
    lo = (x.astype(np.float64) - hi.astype(np.float64)).astype(np.float16)
    return hi, lo


def _prep_sample(a_live, b_live, n_pad, m_pad):
    """Build augmented operand matrices for one sample.

    Returns (A [52, n_pad] f16, B [52, m_pad] f16) so that
    (A.T @ B)[n, m] = 2*a.b - |a|^2 - |b|^2   (= -d2, ~fp32 precision),
    with padded rows/cols pushed to ~-HUGE.
    """
    n_live, d = a_live.shape
    m_live = b_live.shape[0]
    assert d == 16

    a_hi, a_lo = _split16(a_live)
    b_hi, b_lo = _split16(b_live)
    a2 = np.sum(a_live.astype(np.float64) ** 2, axis=1)
    b2 = np.sum(b_live.astype(np.float64) ** 2, axis=1)
    a2n_hi, a2n_lo = _split16(-a2)
    b2_hi, b2_lo = _split16(b2)

    A = np.zeros((52, n_pad), np.float16)
    A[0:16, :n_live] = (np.float16(2) * a_hi).T
    A[16:32, :n_live] = (np.float16(2) * a_lo).T
    A[32:48, :n_live] = (np.float16(2) * a_hi).T
    A[48, :] = np.float16(-1)
    A[49, :] = np.float16(-1)
    A[50, :n_live] = a2n_hi
    A[50, n_live:] = np.float16(-HUGE)
    A[51, :n_live] = a2n_lo

    B = np.zeros((52, m_pad), np.float16)
    B[0:16, :m_live] = b_hi.T
    B[16:32, :m_live] = b_hi.T
    B[32:48, :m_live] = b_lo.T
    B[48, :m_live] = b2_hi
    B[48, m_live:] = np.float16(HUGE)
    B[49, :m_live] = b2_lo
    B[50, :] = np.float16(1)
    B[51, :] = np.float16(1)
    return A, B


def kernel(o_weights, outputs, t_weights, targets):
    from concourse.bass_utils import run_bass_kernel_spmd

    o_weights = np.asarray(o_weights, np.float32)
    t_weights = np.asarray(t_weights, np.float32)
    outputs = np.asarray(outputs, np.float32)
    targets = np.asarray(targets, np.float32)

    B, N, D = outputs.shape
    M = targets.shape[1]
    assert B % NCORES == 0, f"batch {B} not divisible by {NCORES}"
    n_samples = B // NCORES

    o_idx = [np.nonzero(o_weights[b])[0] for b in range(B)]
    t_idx = [np.nonzero(t_weights[b])[0] for b in range(B)]
    max_n = max(1, max(len(ix) for ix in o_idx))
    max_m = max(1, max(len(ix) for ix in t_idx))
    nt = math.ceil(max_n / 128)
    n_pad = nt * 128
    m_pad = 64 * math.ceil(max_m / 64)

    key = (nt, m_pad, n_samples)
    if key not in _PROGRAM_CACHE:
        _PROGRAM_CACHE[key] = _build_program(nt, m_pad, n_samples)
    nc = _PROGRAM_CACHE[key]

    a_aug = np.zeros((B, 52, n_pad), np.float16)
    b_aug = np.zeros((B, 52, m_pad), np.float16)
    w_arr = np.zeros((B, 128, 32), np.float32)
    nblk = math.ceil(m_pad / 128)
    for b in range(B):
        n_live, m_live = len(o_idx[b]), len(t_idx[b])
        a_aug[b], b_aug[b] = _prep_sample(
            outputs[b][o_idx[b]], targets[b][t_idx[b]], n_pad, m_pad)
        nn = np.arange(n_pad) < n_live
        w_arr[b, :, 0:nt] = nn.reshape(nt, 128).T
        mm = np.zeros(nblk * 128, bool)
        mm[:m_pad] = np.arange(m_pad) < m_live
        w_arr[b, :, 16:16 + nblk] = mm.reshape(nblk, 128).T

    ident = np.eye(128, dtype=np.float16)
    ones = np.ones((128, 1), np.float32)
    in_maps = []
    for k in range(NCORES):
        sl = slice(k * n_samples, (k + 1) * n_samples)
        in_maps.append({
            "a_aug": a_aug[sl], "b_aug": b_aug[sl], "w": w_arr[sl],
            "ident": ident, "ones": ones,
        })

    trace = bool(os.environ.get("CHAMFER_TRACE"))
    kw = {}
    if trace:
        kw = {"trace": True,
              "tmpdir": os.environ.get("CHAMFER_TRACE_DIR") or None}
    res = run_bass_kernel_spmd(nc, in_maps, list(range(NCORES)), **kw)
    if trace and res.exec_time_ns is not None:
        print(f"HW exec time: {res.exec_time_ns} ns")

    out = np.empty((B,), np.float32)
    for k in range(NCORES):
        out[k * n_samples:(k + 1) * n_samples] = res.results[k]["y"][0]
    return out
